# revision 20
# baseline (speedup 1.0000x reference)
"""Trainium2 Bass kernel for DiffusionCoordinateInitializer.

Math: target = latent @ W + b            ([B*N, 1024] @ [1024, 3])
      scan:  x <- a*x + (1-a)*target  over alphas = (steps..1)/steps, x0 = noise
Closed form: x_final = P*noise + (1-P)*target,  P = prod(t/steps) = steps!/steps^steps.
For steps=50, P ~= 3e-21: invisible at fp32. The P-term is folded in on the
host (exact for any steps); the device computes only target.

Strategy (pure data parallel over the 32768 rows, 4096 rows/core on 8 cores):
  - Host casts latent to bf16 (halves HBM traffic; rel err ~2e-3 vs the 2e-2
    gate) and pre-transposes it so the device needs no PE transposes:
    chunk[p][j*nr+c] = lat[r0+c, j*128+p]  (nr rows per chunk, j = d block).
  - Device streams 7 x 1 MiB chunks + 4 x 256 KiB tail chunks (contiguous
    per-partition DMAs on the sync HWDGE ring); each chunk is chased by 8
    accumulating bf16 matmuls lhsT=W_j [128,3] into its own PSUM bank.
    Tail chunks are small so the post-last-byte critical path is one short
    N=128 matmul burst instead of a full N=512 one.
  - Early gpsimd memset feeds 9 HAM-warmup matmuls, and 2 keepalive matmuls
    after each chunk pin the PE clock gate at K=8/8 across DMA gaps.
  - Bias is added during the PSUM->SBUF copy (DVE tensor_scalar_add with a
    [3,1] per-partition scalar); per-chunk [3,nr] results ship on the scalar
    HWDGE ring so output DMAs never block input chunks.
"""

import os
import sys

for _p in ("/opt/trn_rl_repo", "/root/.axon_site/_ro/trn_rl_repo"):
    if os.path.isdir(_p):
        if _p not in sys.path:
            sys.path.insert(0, _p)
        break

from contextlib import ExitStack

import ml_dtypes
import numpy as np

import concourse.bacc as bacc
import concourse.bass as bass
import concourse.mybir as mybir
import concourse.tile as tile
from concourse.bass_utils import run_bass_kernel_spmd

F32 = mybir.dt.float32
BF16 = mybir.dt.bfloat16
BF16_NP = ml_dtypes.bfloat16

NCORES = 8
B, N, D, K = 4, 8192, 1024, 3
R_TOTAL = B * N             # 32768 rows
R_CORE = R_TOTAL // NCORES  # 4096 rows per core
RG = 512                    # rows per main chunk (= one PSUM bank of f32)
NG = 7                      # main chunks per core
TAILS = (256, 128, 128)     # tail chunk sizes (7*512 + 512 = 4096)
NJ = D // 128               # 8 d-blocks of 128
N_WARM = 9
N_KEEP = 2

_BUILT = None


def _build():
    global _BUILT
    if _BUILT is not None:
        return _BUILT

    nc = bacc.Bacc(
        "TRN2", debug=False, target_bir_lowering=False, num_devices=NCORES
    )

    latm = nc.dram_tensor(
        "latm", [NG, 128, NJ * RG], BF16, kind="ExternalInput"
    ).ap()
    latqs = [
        nc.dram_tensor(
            f"latq{i}", [128, NJ * t], BF16, kind="ExternalInput"
        ).ap()
        for i, t in enumerate(TAILS)
    ]
    wb = nc.dram_tensor("wb", [128, NJ * K], BF16, kind="ExternalInput").ap()
    bb = nc.dram_tensor("bb", [K, 1], F32, kind="ExternalInput").ap()
    outT = nc.dram_tensor("outT", [K, R_CORE], F32, kind="ExternalOutput").ap()

    with tile.TileContext(nc) as tc, ExitStack() as ctx:
        consts = ctx.enter_context(tc.tile_pool(name="consts", bufs=1))
        latp = ctx.enter_context(tc.tile_pool(name="latp", bufs=2 * NG))
        latpq = ctx.enter_context(tc.tile_pool(name="latpq", bufs=1))
        psp = ctx.enter_context(tc.tile_pool(name="psp", bufs=7, space="PSUM"))
        pswarm = ctx.enter_context(tc.tile_pool(name="pswarm", bufs=1, space="PSUM"))
        outp = ctx.enter_context(tc.tile_pool(name="outp", bufs=1))

        wb_sb = consts.tile([128, NJ * K], BF16)
        nc.gpsimd.dma_start(out=wb_sb[:], in_=wb)
        b_sb = consts.tile([K, 1], F32)
        nc.gpsimd.dma_start(out=b_sb[:], in_=bb)
        out_sb = outp.tile([K, R_CORE], F32)

        # HAM warmup: gpsimd memset is ready early, so the PE can start its
        # clock-gate warmup matmuls right at block entry while the first
        # input chunk is still streaming.
        warm = consts.tile([128, RG], BF16)
        nc.gpsimd.memset(warm[:], 0)
        ps_warm = pswarm.tile([128, RG], F32, tag="warm")
        for _ in range(N_WARM):
            nc.tensor.matmul(
                ps_warm[:], warm[:, 0:128], warm[:], start=True, stop=True
            )

        half = NJ // 2 * RG
        lat_g = []
        for g in range(NG):
            ta = latp.tile([128, half], BF16, tag="lat")
            nc.sync.dma_start(out=ta[:], in_=latm[g, :, 0:half])
            tb = latp.tile([128, half], BF16, tag="lat")
            nc.sync.dma_start(out=tb[:], in_=latm[g, :, half : NJ * RG])
            lat_g.append((ta, tb))
        lat_q = []
        for i, t_rows in enumerate(TAILS):
            t = latpq.tile([128, NJ * t_rows], BF16, tag=f"latq{i}")
            nc.sync.dma_start(out=t[:], in_=latqs[i])
            lat_q.append(t)

        def chunk(rhs_of_j, nr, col0, keepalive, ka_rhs=None):
            psO = psp.tile([K, RG], F32, tag="ps")
            for j in range(NJ):
                nc.tensor.matmul(
                    psO[:, 0:nr],
                    wb_sb[:, bass.ts(j, K)],
                    rhs_of_j(j),
                    start=(j == 0),
                    stop=(j == NJ - 1),
                )
            if keepalive:
                # keep the PE HAM-warm across the DMA gap to the next chunk
                # (reads this chunk's tile, so it schedules here)
                for _ in range(N_KEEP):
                    nc.tensor.matmul(
                        ps_warm[0:K, :],
                        wb_sb[:, 0:K],
                        ka_rhs,
                        start=True,
                        stop=True,
                    )
            nc.vector.tensor_scalar_add(
                out_sb[:, col0 : col0 + nr], psO[:, 0:nr], b_sb[:]
            )

        nh = NJ // 2
        for g in range(NG):
            ta, tb = lat_g[g]
            chunk(
                lambda j, ta=ta, tb=tb: (
                    ta[:, j * RG : (j + 1) * RG]
                    if j < nh
                    else tb[:, (j - nh) * RG : (j - nh + 1) * RG]
                ),
                RG,
                g * RG,
                keepalive=(g < NG - 1),
                ka_rhs=ta[:, 0:RG],
            )
        # one output DMA for everything computed so far; the short tail gets
        # its own small final DMA so the last transfer is cheap
        nc.scalar.dma_start(
            out=outT[:, 0 : NG * RG], in_=out_sb[:, 0 : NG * RG]
        )
        col = NG * RG
        for i, t_rows in enumerate(TAILS):
            rhs = lat_q[i]
            chunk(
                lambda j, rhs=rhs, t=t_rows: rhs[:, j * t : (j + 1) * t],
                t_rows,
                col,
                keepalive=False,
            )
            col += t_rows
        nc.scalar.dma_start(
            out=outT[:, NG * RG : R_CORE], in_=out_sb[:, NG * RG : R_CORE]
        )

    nc.compile()
    _BUILT = nc
    return nc


def _prep_inputs(latent, W, b):
    lat_bf = np.asarray(latent, np.float32).reshape(R_TOTAL, D).astype(BF16_NP)
    lat_bf = lat_bf.reshape(NCORES, R_CORE, D)
    rmain = NG * RG  # 3584
    # latm[core, g, p, j*RG + c] = lat[core, g*512 + c, j*128 + p]
    latm = np.ascontiguousarray(
        lat_bf[:, :rmain].reshape(NCORES, NG, RG, NJ, 128).transpose(0, 1, 4, 3, 2)
    ).reshape(NCORES, NG, 128, NJ * RG)
    # latq{i}[core, p, j*t + c] = lat[core, tail_r0 + c, j*128 + p]
    latq = []
    r0 = rmain
    for t_rows in TAILS:
        blk = np.ascontiguousarray(
            lat_bf[:, r0 : r0 + t_rows]
            .reshape(NCORES, t_rows, NJ, 128)
            .transpose(0, 3, 2, 1)
        ).reshape(NCORES, 128, NJ * t_rows)
        latq.append(blk)
        r0 += t_rows
    wbm = np.ascontiguousarray(
        np.asarray(W, np.float32)
        .reshape(NJ, 128, K)
        .transpose(1, 0, 2)
        .reshape(128, NJ * K)
        .astype(BF16_NP)
    )
    bbm = np.asarray(b, np.float32).reshape(K, 1)
    maps = []
    for c in range(NCORES):
        m = {"latm": latm[c], "wb": wbm, "bb": bbm}
        for i in range(len(TAILS)):
            m[f"latq{i}"] = latq[i][c]
        maps.append(m)
    return maps


def run(latent, W, b, noise, steps, trace=False, tmpdir=None):
    """Returns (output [4,8192,3], BassKernelResults)."""
    nc = _build()
    in_maps = _prep_inputs(latent, W, b)
    res = run_bass_kernel_spmd(
        nc, in_maps, core_ids=list(range(NCORES)), trace=trace, tmpdir=tmpdir
    )
    out = np.concatenate(
        [res.results[c]["outT"].T for c in range(NCORES)], axis=0
    )  # [32768, 3] fp32
    out = out.reshape(B, N, K)

    steps_i = int(steps)
    P = float(np.prod(np.arange(1, steps_i + 1, dtype=np.float64) / steps_i))
    if P > 1e-12:
        out = (np.float32(1.0 - P) * out + np.float32(P) * np.asarray(
            noise, np.float32
        )).astype(np.float32)
    return out, res


def kernel(latent, W, b, noise, steps):
    out, _ = run(latent, W, b, noise, steps)
    return out


# revision 21
# speedup vs baseline: 1.0544x; 1.0544x over previous
"""Trainium2 Bass kernel for DiffusionCoordinateInitializer.

Math: target = latent @ W + b            ([B*N, 1024] @ [1024, 3])
      scan:  x <- a*x + (1-a)*target  over alphas = (steps..1)/steps, x0 = noise
Closed form: x_final = P*noise + (1-P)*target,  P = prod(t/steps) = steps!/steps^steps.
For steps=50, P ~= 3e-21: invisible at fp32. The P-term is folded in on the
host (exact for any steps); the device computes only target.

Strategy (pure data parallel over the 32768 rows, 4096 rows/core on 8 cores):
  - Host casts latent to bf16 (halves HBM traffic; rel err ~2e-3 vs the 2e-2
    gate) and pre-transposes it so the device needs no PE transposes:
    chunk[p][j*nr+c] = lat[r0+c, j*128+p]  (nr rows per chunk, j = d block).
  - Device streams 7 x 1 MiB chunks + 4 x 256 KiB tail chunks (contiguous
    per-partition DMAs on the sync HWDGE ring); each chunk is chased by 8
    accumulating bf16 matmuls lhsT=W_j [128,3] into its own PSUM bank.
    Tail chunks are small so the post-last-byte critical path is one short
    N=128 matmul burst instead of a full N=512 one.
  - Early gpsimd memset feeds 9 HAM-warmup matmuls, and 2 keepalive matmuls
    after each chunk pin the PE clock gate at K=8/8 across DMA gaps.
  - Bias is added during the PSUM->SBUF copy (DVE tensor_scalar_add with a
    [3,1] per-partition scalar); per-chunk [3,nr] results ship on the scalar
    HWDGE ring so output DMAs never block input chunks.
"""

import os
import sys

for _p in ("/opt/trn_rl_repo", "/root/.axon_site/_ro/trn_rl_repo"):
    if os.path.isdir(_p):
        if _p not in sys.path:
            sys.path.insert(0, _p)
        break

from contextlib import ExitStack

import ml_dtypes
import numpy as np

import concourse.bacc as bacc
import concourse.bass as bass
import concourse.mybir as mybir
import concourse.tile as tile
from concourse.bass_utils import run_bass_kernel_spmd

F32 = mybir.dt.float32
BF16 = mybir.dt.bfloat16
BF16_NP = ml_dtypes.bfloat16

NCORES = 8
B, N, D, K = 4, 8192, 1024, 3
R_TOTAL = B * N             # 32768 rows
R_CORE = R_TOTAL // NCORES  # 4096 rows per core
RG = 512                    # rows per main chunk (= one PSUM bank of f32)
NG = 7                      # main chunks per core
TAILS = (256, 128, 128)     # tail chunk sizes (7*512 + 512 = 4096)
NJ = D // 128               # 8 d-blocks of 128
N_WARM = 9
N_KEEP = 2

_BUILT = None


def _build():
    global _BUILT
    if _BUILT is not None:
        return _BUILT

    nc = bacc.Bacc(
        "TRN2", debug=False, target_bir_lowering=False, num_devices=NCORES
    )

    latm = nc.dram_tensor(
        "latm", [NG, 128, NJ * RG], BF16, kind="ExternalInput"
    ).ap()
    latqs = [
        nc.dram_tensor(
            f"latq{i}", [128, NJ * t], BF16, kind="ExternalInput"
        ).ap()
        for i, t in enumerate(TAILS)
    ]
    wb = nc.dram_tensor("wb", [128, NJ * K], BF16, kind="ExternalInput").ap()
    bb = nc.dram_tensor("bb", [K, 1], F32, kind="ExternalInput").ap()
    outT = nc.dram_tensor("outT", [K, R_CORE], F32, kind="ExternalOutput").ap()

    with tile.TileContext(nc) as tc, ExitStack() as ctx:
        consts = ctx.enter_context(tc.tile_pool(name="consts", bufs=1))
        latp = ctx.enter_context(tc.tile_pool(name="latp", bufs=2 * NG))
        latpq = ctx.enter_context(tc.tile_pool(name="latpq", bufs=1))
        psp = ctx.enter_context(tc.tile_pool(name="psp", bufs=7, space="PSUM"))
        pswarm = ctx.enter_context(tc.tile_pool(name="pswarm", bufs=1, space="PSUM"))
        outp = ctx.enter_context(tc.tile_pool(name="outp", bufs=1))

        wb_sb = consts.tile([128, NJ * K], BF16)
        nc.gpsimd.dma_start(out=wb_sb[:], in_=wb)
        b_sb = consts.tile([K, 1], F32)
        nc.gpsimd.dma_start(out=b_sb[:], in_=bb)
        out_sb = outp.tile([K, R_CORE], F32)

        # HAM warmup: gpsimd memset is ready early, so the PE can start its
        # clock-gate warmup matmuls right at block entry while the first
        # input chunk is still streaming.
        warm = consts.tile([128, RG], BF16)
        nc.gpsimd.memset(warm[:], 0)
        ps_warm = pswarm.tile([128, RG], F32, tag="warm")
        for _ in range(N_WARM):
            nc.tensor.matmul(
                ps_warm[:], warm[:, 0:128], warm[:], start=True, stop=True
            )

        half = NJ // 2 * RG
        lat_g = []
        for g in range(NG):
            ta = latp.tile([128, half], BF16, tag="lat")
            nc.sync.dma_start(out=ta[:], in_=latm[g, :, 0:half])
            tb = latp.tile([128, half], BF16, tag="lat")
            nc.sync.dma_start(out=tb[:], in_=latm[g, :, half : NJ * RG])
            lat_g.append((ta, tb))
        lat_q = []
        for i, t_rows in enumerate(TAILS):
            t = latpq.tile([128, NJ * t_rows], BF16, tag=f"latq{i}")
            nc.sync.dma_start(out=t[:], in_=latqs[i])
            lat_q.append(t)

        def chunk(rhs_of_j, nr, col0, keepalive, ka_rhs=None):
            psO = psp.tile([K, RG], F32, tag="ps")
            for j in range(NJ):
                nc.tensor.matmul(
                    psO[:, 0:nr],
                    wb_sb[:, bass.ts(j, K)],
                    rhs_of_j(j),
                    start=(j == 0),
                    stop=(j == NJ - 1),
                )
            if keepalive:
                # keep the PE HAM-warm across the DMA gap to the next chunk
                # (reads this chunk's tile, so it schedules here)
                for _ in range(N_KEEP):
                    nc.tensor.matmul(
                        ps_warm[0:K, :],
                        wb_sb[:, 0:K],
                        ka_rhs,
                        start=True,
                        stop=True,
                    )
            nc.vector.tensor_scalar_add(
                out_sb[:, col0 : col0 + nr], psO[:, 0:nr], b_sb[:]
            )

        nh = NJ // 2
        for g in range(NG):
            ta, tb = lat_g[g]
            chunk(
                lambda j, ta=ta, tb=tb: (
                    ta[:, j * RG : (j + 1) * RG]
                    if j < nh
                    else tb[:, (j - nh) * RG : (j - nh + 1) * RG]
                ),
                RG,
                g * RG,
                keepalive=(g < NG - 1),
                ka_rhs=ta[:, 0:RG],
            )
        # one output DMA for everything computed so far; the short tail gets
        # its own small final DMA so the last transfer is cheap
        nc.gpsimd.dma_start(
            out=outT[:, 0 : NG * RG], in_=out_sb[:, 0 : NG * RG]
        )
        col = NG * RG
        for i, t_rows in enumerate(TAILS):
            rhs = lat_q[i]
            chunk(
                lambda j, rhs=rhs, t=t_rows: rhs[:, j * t : (j + 1) * t],
                t_rows,
                col,
                keepalive=False,
            )
            col += t_rows
        nc.scalar.dma_start(
            out=outT[:, NG * RG : R_CORE], in_=out_sb[:, NG * RG : R_CORE]
        )

    nc.compile()
    _BUILT = nc
    return nc


def _prep_inputs(latent, W, b):
    lat_bf = np.asarray(latent, np.float32).reshape(R_TOTAL, D).astype(BF16_NP)
    lat_bf = lat_bf.reshape(NCORES, R_CORE, D)
    rmain = NG * RG  # 3584
    # latm[core, g, p, j*RG + c] = lat[core, g*512 + c, j*128 + p]
    latm = np.ascontiguousarray(
        lat_bf[:, :rmain].reshape(NCORES, NG, RG, NJ, 128).transpose(0, 1, 4, 3, 2)
    ).reshape(NCORES, NG, 128, NJ * RG)
    # latq{i}[core, p, j*t + c] = lat[core, tail_r0 + c, j*128 + p]
    latq = []
    r0 = rmain
    for t_rows in TAILS:
        blk = np.ascontiguousarray(
            lat_bf[:, r0 : r0 + t_rows]
            .reshape(NCORES, t_rows, NJ, 128)
            .transpose(0, 3, 2, 1)
        ).reshape(NCORES, 128, NJ * t_rows)
        latq.append(blk)
        r0 += t_rows
    wbm = np.ascontiguousarray(
        np.asarray(W, np.float32)
        .reshape(NJ, 128, K)
        .transpose(1, 0, 2)
        .reshape(128, NJ * K)
        .astype(BF16_NP)
    )
    bbm = np.asarray(b, np.float32).reshape(K, 1)
    maps = []
    for c in range(NCORES):
        m = {"latm": latm[c], "wb": wbm, "bb": bbm}
        for i in range(len(TAILS)):
            m[f"latq{i}"] = latq[i][c]
        maps.append(m)
    return maps


def run(latent, W, b, noise, steps, trace=False, tmpdir=None):
    """Returns (output [4,8192,3], BassKernelResults)."""
    nc = _build()
    in_maps = _prep_inputs(latent, W, b)
    res = run_bass_kernel_spmd(
        nc, in_maps, core_ids=list(range(NCORES)), trace=trace, tmpdir=tmpdir
    )
    out = np.concatenate(
        [res.results[c]["outT"].T for c in range(NCORES)], axis=0
    )  # [32768, 3] fp32
    out = out.reshape(B, N, K)

    steps_i = int(steps)
    P = float(np.prod(np.arange(1, steps_i + 1, dtype=np.float64) / steps_i))
    if P > 1e-12:
        out = (np.float32(1.0 - P) * out + np.float32(P) * np.asarray(
            noise, np.float32
        )).astype(np.float32)
    return out, res


def kernel(latent, W, b, noise, steps):
    out, _ = run(latent, W, b, noise, steps)
    return out


# revision 22
# speedup vs baseline: 1.0548x; 1.0004x over previous
"""Trainium2 Bass kernel for DiffusionCoordinateInitializer.

Math: target = latent @ W + b            ([B*N, 1024] @ [1024, 3])
      scan:  x <- a*x + (1-a)*target  over alphas = (steps..1)/steps, x0 = noise
Closed form: x_final = P*noise + (1-P)*target,  P = prod(t/steps) = steps!/steps^steps.
For steps=50, P ~= 3e-21: invisible at fp32. The P-term is folded in on the
host (exact for any steps); the device computes only target.

Strategy (pure data parallel over the 32768 rows, 4096 rows/core on 8 cores):
  - Host casts latent to bf16 (halves HBM traffic; rel err ~2e-3 vs the 2e-2
    gate) and pre-transposes it so the device needs no PE transposes:
    chunk[p][j*nr+c] = lat[r0+c, j*128+p]  (nr rows per chunk, j = d block).
  - Device streams 7 x 1 MiB chunks + 4 x 256 KiB tail chunks (contiguous
    per-partition DMAs on the sync HWDGE ring); each chunk is chased by 8
    accumulating bf16 matmuls lhsT=W_j [128,3] into its own PSUM bank.
    Tail chunks are small so the post-last-byte critical path is one short
    N=128 matmul burst instead of a full N=512 one.
  - Early gpsimd memset feeds 9 HAM-warmup matmuls, and 2 keepalive matmuls
    after each chunk pin the PE clock gate at K=8/8 across DMA gaps.
  - Bias is added during the PSUM->SBUF copy (DVE tensor_scalar_add with a
    [3,1] per-partition scalar); per-chunk [3,nr] results ship on the scalar
    HWDGE ring so output DMAs never block input chunks.
"""

import os
import sys

for _p in ("/opt/trn_rl_repo", "/root/.axon_site/_ro/trn_rl_repo"):
    if os.path.isdir(_p):
        if _p not in sys.path:
            sys.path.insert(0, _p)
        break

from contextlib import ExitStack

import ml_dtypes
import numpy as np

import concourse.bacc as bacc
import concourse.bass as bass
import concourse.mybir as mybir
import concourse.tile as tile
from concourse.bass_utils import run_bass_kernel_spmd

F32 = mybir.dt.float32
BF16 = mybir.dt.bfloat16
BF16_NP = ml_dtypes.bfloat16

NCORES = 8
B, N, D, K = 4, 8192, 1024, 3
R_TOTAL = B * N             # 32768 rows
R_CORE = R_TOTAL // NCORES  # 4096 rows per core
RG = 512                    # rows per main chunk (= one PSUM bank of f32)
NG = 7                      # main chunks per core
TAILS = (256, 128, 128)     # tail chunk sizes (7*512 + 512 = 4096)
NJ = D // 128               # 8 d-blocks of 128
N_WARM = 9
N_KEEP = 2

_BUILT = None


def _build():
    global _BUILT
    if _BUILT is not None:
        return _BUILT

    nc = bacc.Bacc(
        "TRN2", debug=False, target_bir_lowering=False, num_devices=NCORES
    )

    latm = nc.dram_tensor(
        "latm", [NG, 128, NJ * RG], BF16, kind="ExternalInput"
    ).ap()
    latqs = [
        nc.dram_tensor(
            f"latq{i}", [128, NJ * t], BF16, kind="ExternalInput"
        ).ap()
        for i, t in enumerate(TAILS)
    ]
    wb = nc.dram_tensor("wb", [128, NJ * K], BF16, kind="ExternalInput").ap()
    bb = nc.dram_tensor("bb", [K, 1], F32, kind="ExternalInput").ap()
    outT = nc.dram_tensor("outT", [K, R_CORE], F32, kind="ExternalOutput").ap()

    with tile.TileContext(nc) as tc, ExitStack() as ctx:
        consts = ctx.enter_context(tc.tile_pool(name="consts", bufs=1))
        latp = ctx.enter_context(tc.tile_pool(name="latp", bufs=2 * NG))
        latpq = ctx.enter_context(tc.tile_pool(name="latpq", bufs=1))
        psp = ctx.enter_context(tc.tile_pool(name="psp", bufs=7, space="PSUM"))
        pswarm = ctx.enter_context(tc.tile_pool(name="pswarm", bufs=1, space="PSUM"))
        outp = ctx.enter_context(tc.tile_pool(name="outp", bufs=1))

        wb_sb = consts.tile([128, NJ * K], BF16)
        nc.gpsimd.dma_start(out=wb_sb[:], in_=wb)
        b_sb = consts.tile([K, 1], F32)
        nc.gpsimd.dma_start(out=b_sb[:], in_=bb)
        out_main = outp.tile([K, NG * RG], F32)
        out_tail = outp.tile([K, R_CORE - NG * RG], F32, tag="tail")

        # HAM warmup: gpsimd memset is ready early, so the PE can start its
        # clock-gate warmup matmuls right at block entry while the first
        # input chunk is still streaming.
        warm = consts.tile([128, RG], BF16)
        nc.gpsimd.memset(warm[:], 0)
        ps_warm = pswarm.tile([128, RG], F32, tag="warm")
        for _ in range(N_WARM):
            nc.tensor.matmul(
                ps_warm[:], warm[:, 0:128], warm[:], start=True, stop=True
            )

        half = NJ // 2 * RG
        lat_g = []
        for g in range(NG):
            ta = latp.tile([128, half], BF16, tag="lat")
            nc.sync.dma_start(out=ta[:], in_=latm[g, :, 0:half])
            tb = latp.tile([128, half], BF16, tag="lat")
            nc.sync.dma_start(out=tb[:], in_=latm[g, :, half : NJ * RG])
            lat_g.append((ta, tb))
        lat_q = []
        for i, t_rows in enumerate(TAILS):
            t = latpq.tile([128, NJ * t_rows], BF16, tag=f"latq{i}")
            nc.sync.dma_start(out=t[:], in_=latqs[i])
            lat_q.append(t)

        def chunk(rhs_of_j, nr, col0, keepalive, ka_rhs=None, out_dst=None, dst0=0):
            psO = psp.tile([K, RG], F32, tag="ps")
            for j in range(NJ):
                nc.tensor.matmul(
                    psO[:, 0:nr],
                    wb_sb[:, bass.ts(j, K)],
                    rhs_of_j(j),
                    start=(j == 0),
                    stop=(j == NJ - 1),
                )
            if keepalive:
                # keep the PE HAM-warm across the DMA gap to the next chunk
                # (reads this chunk's tile, so it schedules here)
                for _ in range(N_KEEP):
                    nc.tensor.matmul(
                        ps_warm[0:K, :],
                        wb_sb[:, 0:K],
                        ka_rhs,
                        start=True,
                        stop=True,
                    )
            nc.vector.tensor_scalar_add(
                out_dst[:, col0 - dst0 : col0 - dst0 + nr], psO[:, 0:nr], b_sb[:]
            )

        nh = NJ // 2
        for g in range(NG):
            ta, tb = lat_g[g]
            chunk(
                lambda j, ta=ta, tb=tb: (
                    ta[:, j * RG : (j + 1) * RG]
                    if j < nh
                    else tb[:, (j - nh) * RG : (j - nh + 1) * RG]
                ),
                RG,
                g * RG,
                keepalive=(g < NG - 1),
                ka_rhs=ta[:, 0:RG],
                out_dst=out_main,
            )
        # one output DMA for everything computed so far; the short tail gets
        # its own small final DMA so the last transfer is cheap
        nc.scalar.dma_start(out=outT[:, 0 : NG * RG], in_=out_main[:])
        col = NG * RG
        for i, t_rows in enumerate(TAILS):
            rhs = lat_q[i]
            chunk(
                lambda j, rhs=rhs, t=t_rows: rhs[:, j * t : (j + 1) * t],
                t_rows,
                col,
                keepalive=False,
                out_dst=out_tail,
                dst0=NG * RG,
            )
            col += t_rows
        nc.scalar.dma_start(
            out=outT[:, NG * RG : R_CORE], in_=out_tail[:]
        )

    nc.compile()
    _BUILT = nc
    return nc


def _prep_inputs(latent, W, b):
    lat_bf = np.asarray(latent, np.float32).reshape(R_TOTAL, D).astype(BF16_NP)
    lat_bf = lat_bf.reshape(NCORES, R_CORE, D)
    rmain = NG * RG  # 3584
    # latm[core, g, p, j*RG + c] = lat[core, g*512 + c, j*128 + p]
    latm = np.ascontiguousarray(
        lat_bf[:, :rmain].reshape(NCORES, NG, RG, NJ, 128).transpose(0, 1, 4, 3, 2)
    ).reshape(NCORES, NG, 128, NJ * RG)
    # latq{i}[core, p, j*t + c] = lat[core, tail_r0 + c, j*128 + p]
    latq = []
    r0 = rmain
    for t_rows in TAILS:
        blk = np.ascontiguousarray(
            lat_bf[:, r0 : r0 + t_rows]
            .reshape(NCORES, t_rows, NJ, 128)
            .transpose(0, 3, 2, 1)
        ).reshape(NCORES, 128, NJ * t_rows)
        latq.append(blk)
        r0 += t_rows
    wbm = np.ascontiguousarray(
        np.asarray(W, np.float32)
        .reshape(NJ, 128, K)
        .transpose(1, 0, 2)
        .reshape(128, NJ * K)
        .astype(BF16_NP)
    )
    bbm = np.asarray(b, np.float32).reshape(K, 1)
    maps = []
    for c in range(NCORES):
        m = {"latm": latm[c], "wb": wbm, "bb": bbm}
        for i in range(len(TAILS)):
            m[f"latq{i}"] = latq[i][c]
        maps.append(m)
    return maps


def run(latent, W, b, noise, steps, trace=False, tmpdir=None):
    """Returns (output [4,8192,3], BassKernelResults)."""
    nc = _build()
    in_maps = _prep_inputs(latent, W, b)
    res = run_bass_kernel_spmd(
        nc, in_maps, core_ids=list(range(NCORES)), trace=trace, tmpdir=tmpdir
    )
    out = np.concatenate(
        [res.results[c]["outT"].T for c in range(NCORES)], axis=0
    )  # [32768, 3] fp32
    out = out.reshape(B, N, K)

    steps_i = int(steps)
    P = float(np.prod(np.arange(1, steps_i + 1, dtype=np.float64) / steps_i))
    if P > 1e-12:
        out = (np.float32(1.0 - P) * out + np.float32(P) * np.asarray(
            noise, np.float32
        )).astype(np.float32)
    return out, res


def kernel(latent, W, b, noise, steps):
    out, _ = run(latent, W, b, noise, steps)
    return out
